# revision 39
# baseline (speedup 1.0000x reference)
# Bass/Trainium2 kernel for nn_LoRARouter (topk_masking).
#
# Reference computes:
#   gated  = pooled @ Wg^T            [B, D]   (B=8192, D=4096)
#   logits = gated  @ Wr^T            [B, 7]
#   probs  = softmax(logits)
#   ranks  = argsort(argsort(-rand_noise))    per [7, B, :8] group
#   out[m,b,e] = probs[b,m] > 0.5 ? (rank<2)/2 : (rank<1)/1
#
# `gated` is only consumed by the second matmul, so
#   logits = pooled @ (Wr @ Wg)^T
# removing the 275-GFLOP [B,D]x[D,D] matmul. The problem is then at the
# HBM/PE ridge: read pooled (134 MB) + Wg (67 MB once across the fleet).
#
# Design (vs the 173 us small-DMA baseline):
#  - All matmuls run as fp16 two-limb (hi + 2^11-scaled lo) products with
#    fp32 PSUM accumulation: logits = A + B/2048 where A = hi.hi and
#    B = hi.lo' + lo'.hi. Error ~2^-22 per element (measured logit err
#    3e-6 vs min decision margin 5.5e-5), but 1 cycle/row on the PE
#    instead of fp32's 4 -> PE ~65 us/core, under the ~72 us DMA floor.
#  - Host pre-formats every tensor in exact SBUF layout; inputs stream as
#    ~24 big (1-2 MB) DMAs on the sync HWDGE ring instead of ~70 small
#    ones, so the 16 SDMA engines stay fed.
#  - Wg streams first; Weff = Wr@Wg[:,shard] is computed per half-shard
#    and AllGather'd in 2 pipelined collectives triggered at ~13/~25 us,
#    hiding collective latency under the pooled_hidden stream. Each
#    core's own-shard logits run while the first gather is in flight.
#  - Select tail vectorized: per-module thresholds broadcast to
#    [128,448] once, then two full-width ops (was 56 tiny ops).
#
# Sharding (8 cores):
#  - pooled_hidden, rand_noise, output: batch-sharded (1024 rows/core)
#  - Wg: column-sharded (512 output dims/core); AllGather of WeffT.

import numpy as np

import concourse.bass as bass
import concourse.bacc as bacc
import concourse.mybir as mybir
import concourse.tile as tile
from concourse.bass_utils import run_bass_kernel_spmd

F32 = mybir.dt.float32
F16 = mybir.dt.float16
N_CORES = 8
B, D, NM, NE = 8192, 4096, 7, 8      # batch, d_model, n_modules, n_experts
BS = B // N_CORES                    # 1024 batch rows per core
SH = D // N_CORES                    # 512 Weff output dims per core
NK = D // 128                        # 32 contraction chunks of 128
NBC = BS // 128                      # 8 batch chunks of 128 per core
GRP = NM * NE                        # 56 columns per batch chunk
W = NBC * GRP                        # 448 free columns in [128, 448] tiles
SCALE = 2048.0                       # lo-limb scale 2^11
XPC = 16                             # x DMA pieces (2 chunk-slots each)
WPC = 8                              # wg DMA pieces (8 e-chunks each)

ALU = mybir.AluOpType
AF = mybir.ActivationFunctionType

_CACHE = {}
LAST_RESULTS = None  # test harness introspection


# Uniform chunk-slot order (identical on every core — the program is SPMD
# shared): half-0 chunks of all cores, then half-1. Slot s holds global
# chunk 4*b + 2*h + c with h = s//16, b = (s%16)//2, c = s%2; this matches
# the gathered WeffT block layout exactly.
CHUNK_ORDER = [4 * b + 2 * h + c for h in (0, 1) for b in range(N_CORES)
               for c in (0, 1)]


def _build_program():
    nc = bacc.Bacc(
        "TRN2", target_bir_lowering=False, debug=False, num_devices=N_CORES
    )

    # x limbs as XPC contiguous 1-MB pieces: piece p rows [128p,128(p+1)),
    # piece = 2 chunk-slots, slot s (2048 cols) = [hi:1024 b][lo:1024 b]
    xi = nc.dram_tensor("xi", [XPC * 128, 2 * 2048], F16, kind="ExternalInput")
    # wg limbs as WPC contiguous pieces: [h][ec 0..31][hi:256 d][lo:256 d]
    wgi = nc.dram_tensor("wgi", [WPC * 128, 8 * 512], F16, kind="ExternalInput")
    wrh = nc.dram_tensor("wrh", [128, NK * NM], F16, kind="ExternalInput")
    wrl = nc.dram_tensor("wrl", [128, NK * NM], F16, kind="ExternalInput")
    nzin = nc.dram_tensor("nz", [128, W], F32, kind="ExternalInput")
    cstin = nc.dram_tensor("cst", [128, W], F32, kind="ExternalInput")
    o = nc.dram_tensor("o", [128, W], F32, kind="ExternalOutput")

    # AllGather bounce per half, carrying PRE-SPLIT fp16 limbs (the split
    # runs on own data before the gather, off the critical path, so the
    # gathered weights are matmul-ready with no post-gather DVE work):
    # wfin[h][p, 0:14] = hi limbs (c=0,1), [p, 14:28] = lo limbs.
    wfin = [nc.dram_tensor(f"wfin{h}", [128, 4 * NM], F16) for h in (0, 1)]
    wfout = [
        nc.dram_tensor(
            f"wfout{h}", [N_CORES * 128, 4 * NM], F16, addr_space="Shared"
        )
        for h in (0, 1)
    ]


    with tile.TileContext(nc) as tc:
        with (
            tc.tile_pool(name="wgp", bufs=4) as wgp,
            tc.tile_pool(name="xp", bufs=XPC) as xp,
            tc.tile_pool(name="sp", bufs=1) as sp,
            tc.tile_pool(name="scr", bufs=2) as scp,
            tc.tile_pool(name="sm", bufs=16) as smp,
            tc.tile_pool(name="pw", bufs=2, space="PSUM") as pw,
            tc.tile_pool(name="pl", bufs=4, space="PSUM") as pl,
            tc.tile_pool(name="pt", bufs=2, space="PSUM") as pt,
        ):
            # ---- small input DMAs on the scalar HWDGE ring ----
            wrh_sb = sp.tile([128, NK * NM], F16, tag="wrh")
            wrl_sb = sp.tile([128, NK * NM], F16, tag="wrl")
            nz = sp.tile([128, W], F32, tag="nz")
            cstt = sp.tile([128, W], F32, tag="cst")
            nc.scalar.dma_start(wrh_sb[:], wrh[:])
            nc.scalar.dma_start(wrl_sb[:], wrl[:])
            nc.scalar.dma_start(nz[:], nzin[:])
            nc.scalar.dma_start(cstt[:], cstin[:])
            # warm the Exp activation table while the ring is idle
            warm = smp.tile([128, 1], F32, tag="warm")
            nc.scalar.activation(warm[:], nz[:, :1], AF.Exp)

            # identity for PE transposes (only [:7,:7] used)
            ident = sp.tile([128, 128], F32, tag="ident")
            from concourse.masks import make_identity
            make_identity(nc, ident[:])

            # ---- big input streams on the sync HWDGE ring (wg first) ----
            wgt = []  # piece wp: h = wp//4, ec in [8*(wp%4), 8*(wp%4)+8)
            for wp in range(WPC):
                t = wgp.tile([128, 8 * 512], F16, tag="wg")
                nc.sync.dma_start(t[:], wgi[wp * 128:(wp + 1) * 128, :])
                wgt.append(t)
            xts = []  # piece p: chunk-slots 2p, 2p+1
            for p in range(XPC):
                t = xp.tile([128, 2 * 2048], F16, tag="x")
                nc.sync.dma_start(t[:], xi[p * 128:(p + 1) * 128, :])
                xts.append(t)

            # ---- Weff halves: [7,256] = Wr @ Wg[:, own 256 cols], two-limb
            # chains A (hi.hi) and B (hi.lo' + lo'.hi), then AllGather ----
            wfg16 = []     # gathered limbs (hi, lo) per half
            for h in (0, 1):
                wA = pw.tile([7, 256], F32, tag="pw", name=f"wA{h}")
                wB = pw.tile([7, 256], F32, tag="pw", name=f"wB{h}")
                for ec in range(NK):
                    wp = h * 4 + ec // 8
                    base = (ec % 8) * 512
                    ghi = wgt[wp][:, base:base + 256]
                    glo = wgt[wp][:, base + 256:base + 512]
                    rsl = slice(ec * NM, (ec + 1) * NM)
                    st, sp_ = (ec == 0), (ec == NK - 1)
                    nc.tensor.matmul(wA[:], wrh_sb[:, rsl], ghi, start=st, stop=sp_)
                    nc.tensor.matmul(wB[:], wrl_sb[:, rsl], ghi, start=st, stop=False)
                    nc.tensor.matmul(wB[:], wrh_sb[:, rsl], glo, start=False, stop=sp_)
                # combine: wf = wA + wB/SCALE  [7, 256] f32
                tb = scp.tile([7, 256], F32, tag="scr2")
                nc.vector.tensor_scalar_mul(tb[:], wB[:], 1.0 / SCALE)
                wf = sp.tile([7, 256], F32, tag=f"wf{h}")
                nc.vector.tensor_tensor(wf[:], tb[:], wA[:], ALU.add)
                # transpose both 128-col blocks to d-major [128, 14]
                wsh = sp.tile([128, 2 * NM], F32, tag=f"wsh{h}")
                for c in (0, 1):
                    tr = pt.tile([128, NM], F32, tag="pt")
                    nc.tensor.transpose(
                        tr[:], wf[:, c * 128:(c + 1) * 128], ident[:7, :7]
                    )
                    nc.vector.tensor_copy(wsh[:, c * NM:(c + 1) * NM], tr[:])
                # split own limbs BEFORE the gather (hidden under the DMA
                # stream), then bounce both limb sets for this half
                whi = sp.tile([128, 2 * NM], F16, tag=f"whi{h}")
                wlo = sp.tile([128, 2 * NM], F16, tag=f"wlo{h}")
                hi32 = scp.tile([128, 2 * NM], F32, tag="scr4")
                dif = scp.tile([128, 2 * NM], F32, tag="scr4")
                nc.vector.tensor_copy(whi[:], wsh[:])
                nc.vector.tensor_copy(hi32[:], whi[:])
                nc.vector.tensor_tensor(dif[:], wsh[:], hi32[:], ALU.subtract)
                nc.vector.tensor_scalar_mul(wlo[:], dif[:], SCALE)
                nc.scalar.dma_start(wfin[h][:, :2 * NM], whi[:])
                nc.scalar.dma_start(wfin[h][:, 2 * NM:], wlo[:])

            # pipelined AllGathers (both triggers before either load-back
            # so a slow first gather can't delay the second trigger)
            for h in (0, 1):
                nc.gpsimd.collective_compute(
                    "AllGather",
                    ALU.bypass,
                    replica_groups=[list(range(N_CORES))],
                    ins=[wfin[h][:]],
                    outs=[wfout[h][:]],
                )
            # load back matmul-ready limbs: wfg[h][p, j*28 + (hi|lo)*14 + c*7+m].
            # On the scalar HWDGE ring: the rearrange generates ~1024 tiny
            # descriptors, which SWDGE's Q7 software emits serially (~4-6 us)
            # but HWDGE RTL emits at line rate — this load sits right on the
            # gather -> first-matmul critical path.
            for h in (0, 1):
                wfg = sp.tile([128, N_CORES * 4 * NM], F16, tag=f"wfg{h}")
                nc.scalar.dma_start(
                    wfg[:].rearrange("p (j f) -> p j f", j=N_CORES),
                    wfout[h][:].rearrange("(j p) f -> p j f", p=128),
                )
                wfg16.append(wfg)

            # ---- expert ranks from rand_noise (overlaps the DMA phase on
            # DVE). r[e] = #{j<e: v_j >= v_e} + #{j>e: v_j > v_e}; acc
            # starts at cst[e] = 7-e; each offset's comparison adds 1 at
            # the A position and subtracts 1 at the B position. ----
            acc = sp.tile([128, W], F32, tag="acc")
            nc.vector.tensor_copy(acc[:], cstt[:])
            nz_r = nz[:].rearrange("p (c m e) -> p c m e", m=NM, e=NE)
            acc_r = acc[:].rearrange("p (c m e) -> p c m e", m=NM, e=NE)
            for off in range(1, NE):
                wdt = NE - off
                scr = scp.tile([128, NBC * NM * 7], F32, tag="scr")
                scr_v = scr[:, : NBC * NM * wdt].rearrange(
                    "p (c m e) -> p c m e", m=NM, e=wdt
                )
                nc.vector.tensor_tensor(
                    scr_v, nz_r[:, :, :, 0:wdt], nz_r[:, :, :, off:NE], ALU.is_ge
                )
                nc.vector.tensor_tensor(
                    acc_r[:, :, :, off:NE], acc_r[:, :, :, off:NE], scr_v, ALU.add
                )
                nc.vector.tensor_tensor(
                    acc_r[:, :, :, 0:wdt], acc_r[:, :, :, 0:wdt], scr_v, ALU.subtract
                )

            # ---- logitsT: two-limb chains per batch half, accumulated
            # over the 32 chunk-slots in stream order ----
            psA = [pl.tile([7, 512], F32, tag="pl", name=f"pA{bh}") for bh in (0, 1)]
            psB = [pl.tile([7, 512], F32, tag="pl", name=f"pB{bh}") for bh in (0, 1)]

            def slot_lhsT(s):
                # weight limb slices [128, 7] for chunk-slot s; slot order
                # matches the gathered block layout (see CHUNK_ORDER)
                h = s // 16
                j, c = (s % 16) // 2, s % 2
                wfg = wfg16[h]
                base = j * 4 * NM
                hi_sl = slice(base + c * NM, base + (c + 1) * NM)
                lo_sl = slice(base + 2 * NM + c * NM, base + 2 * NM + (c + 1) * NM)
                return wfg[:, hi_sl], wfg[:, lo_sl]

            # Batch-half-MAJOR chains: bh=0's full 32-slot chain runs first,
            # so its combine/softmax/select tail overlaps under bh=1's
            # ~21 us of matmuls — only half the tail stays exposed after
            # the last matmul (was: both PSUM chains stopped on the final
            # matmul, serializing the whole tail behind it).
            logT = sp.tile([7, BS], F32, tag="logT")
            thr = sp.tile([128, NBC * NM], F32, tag="thr")
            threp = sp.tile([128, W], F32, tag="threp")
            valrep = sp.tile([128, W], F32, tag="valrep")
            outt = sp.tile([128, W], F32, tag="outt")
            thr_v = thr[:].rearrange("p (c m) -> p c m", m=NM)
            threp_v = threp[:].rearrange("p (c m e) -> p c m e", m=NM, e=NE)
            HBC = NBC // 2          # 4 batch chunks per half
            HW_ = W // 2            # 224 select columns per half
            for bh in (0, 1):
                for s in range(NK):
                    xt = xts[s // 2]
                    xb = (s % 2) * 2048
                    whi, wlo = slot_lhsT(s)
                    st, sp_ = (s == 0), (s == NK - 1)
                    xhi = xt[:, xb + bh * 512: xb + bh * 512 + 512]
                    xlo = xt[:, xb + 1024 + bh * 512: xb + 1024 + bh * 512 + 512]
                    nc.tensor.matmul(psA[bh][:], whi, xhi, start=st, stop=sp_)
                    nc.tensor.matmul(psB[bh][:], wlo, xhi, start=st, stop=False)
                    nc.tensor.matmul(psB[bh][:], whi, xlo, start=False, stop=sp_)
                # combine: logT half = A + B/SCALE
                tb = scp.tile([7, 512], F32, tag="scr5")
                nc.vector.tensor_scalar_mul(tb[:], psB[bh][:], 1.0 / SCALE)
                nc.vector.tensor_tensor(
                    logT[:, bh * 512:(bh + 1) * 512], tb[:], psA[bh][:], ALU.add
                )
                # softmax>0.5 -> thr in {1,2} for this half's batch chunks
                for bc in range(bh * HBC, (bh + 1) * HBC):
                    plt = pt.tile([128, NM], F32, tag="pt")
                    nc.tensor.transpose(
                        plt[:], logT[:, bc * 128:(bc + 1) * 128], ident[:7, :7]
                    )
                    # |logits| < 9: exp cannot overflow fp32, skip max-sub
                    ssum = smp.tile([128, 1], F32, tag="ssum")
                    shalf = smp.tile([128, 1], F32, tag="shalf")
                    expt = smp.tile([128, NM], F32, tag="expt")
                    nc.scalar.activation(
                        expt[:], plt[:], AF.Exp, accum_out=ssum[:]
                    )
                    nc.vector.tensor_scalar_mul(shalf[:], ssum[:], 0.5)
                    nc.vector.tensor_scalar(
                        out=thr[:, bc * NM:(bc + 1) * NM], in0=expt[:],
                        scalar1=shalf[:], scalar2=1.0, op0=ALU.is_gt, op1=ALU.add,
                    )
                # select for this half (all on DVE; gpsimd runs these
                # strided ops 4-5x slower)
                cs = slice(bh * HBC, (bh + 1) * HBC)
                ws = slice(bh * HW_, (bh + 1) * HW_)
                for e in range(NE):
                    nc.vector.tensor_copy(threp_v[:, cs, :, e], thr_v[:, cs, :])
                nc.vector.tensor_scalar(
                    out=valrep[:, ws], in0=threp[:, ws], scalar1=-0.5,
                    scalar2=1.5, op0=ALU.mult, op1=ALU.add,
                )
                nc.vector.tensor_tensor(
                    outt[:, ws], acc[:, ws], threp[:, ws], ALU.is_lt
                )
                nc.vector.tensor_tensor(
                    outt[:, ws], outt[:, ws], valrep[:, ws], ALU.mult
                )
            nc.scalar.dma_start(o[:], outt[:])

    nc.compile()
    return nc


def _get_program():
    if "nc" not in _CACHE:
        _CACHE["nc"] = _build_program()
    return _CACHE["nc"]


def _split16(x):
    hi = x.astype(np.float16)
    lo = ((x - hi.astype(np.float32)) * SCALE).astype(np.float16)
    return hi, lo


def _const_input():
    base = (7.0 - np.arange(NE, dtype=np.float32))
    return np.ascontiguousarray(
        np.broadcast_to(np.tile(base, NBC * NM), (128, W))
    )


def _prep_core(i, ph, wg_full, rn):
    bsl = slice(i * BS, (i + 1) * BS)
    xc = np.ascontiguousarray(ph[bsl].T).reshape(NK, 128, BS)[CHUNK_ORDER]
    hi, lo = _split16(xc)                          # [32, 128, 1024] each
    xi = np.concatenate([hi, lo], axis=2)          # [32, 128, 2048]
    # piece-contiguous: piece p (2 slots) occupies rows [128p, 128(p+1))
    xi = np.ascontiguousarray(
        xi.reshape(XPC, 2, 128, 2048).transpose(0, 2, 1, 3)
        .reshape(XPC * 128, 2 * 2048)
    )

    esl = slice(i * SH, (i + 1) * SH)
    wgc = np.ascontiguousarray(wg_full[:, esl]).reshape(NK, 128, SH)
    segs = []
    for h in (0, 1):
        seg = wgc[:, :, h * 256:(h + 1) * 256]     # [32, 128, 256]
        shi, slo = _split16(seg)
        segs.append(np.concatenate([shi, slo], axis=2))   # [32, 128, 512]
    wgi = np.stack(segs, 0)                        # [2, 32, 128, 512]
    # piece-contiguous: piece wp = (h, 8-ec block) occupies rows [128wp, ..)
    wgi = np.ascontiguousarray(
        wgi.reshape(2, 4, 8, 128, 512).transpose(0, 1, 3, 2, 4)
        .reshape(WPC * 128, 8 * 512)
    )

    # nz[p, c*56 + m*8 + e] = rn[m, 1024*i + 128*c + p, e]
    nz_i = np.ascontiguousarray(
        rn[:, bsl, :].transpose(1, 0, 2)
        .reshape(NBC, 128, GRP).transpose(1, 0, 2).reshape(128, W)
    )
    return xi, wgi, nz_i


def kernel(pooled_hidden, Wg, Wr, rand_noise):
    global LAST_RESULTS
    ph = np.asarray(pooled_hidden, dtype=np.float32)
    wg_full = np.asarray(Wg, dtype=np.float32)
    wr = np.asarray(Wr, dtype=np.float32)
    rn = np.asarray(rand_noise, dtype=np.float32)

    nc = _get_program()
    cst = _const_input()

    # Wr in SBUF layout [p, ec*7+m] = Wr[m, ec*128+p], fp16 limbs
    wrt = np.ascontiguousarray(
        wr.T.reshape(NK, 128, NM).transpose(1, 0, 2).reshape(128, NK * NM)
    )
    wrh_full, wrl_full = _split16(wrt)
    wrh_full = np.ascontiguousarray(wrh_full)
    wrl_full = np.ascontiguousarray(wrl_full)

    in_maps = []
    for i in range(N_CORES):
        xi, wgi, nz_i = _prep_core(i, ph, wg_full, rn)
        in_maps.append(
            {"xi": xi, "wgi": wgi, "wrh": wrh_full, "wrl": wrl_full,
             "nz": nz_i, "cst": cst}
        )

    res = run_bass_kernel_spmd(nc, in_maps, list(range(N_CORES)))
    LAST_RESULTS = res

    out = np.empty((NM, B, NE), dtype=np.float32)
    for i, r in enumerate(res.results):
        oc = r["o"]  # [128, 448]
        out[:, i * BS:(i + 1) * BS, :] = (
            oc.reshape(128, NBC, NM, NE).transpose(2, 1, 0, 3).reshape(NM, BS, NE)
        )
    return out


# revision 41
# speedup vs baseline: 1.0072x; 1.0072x over previous
# Bass/Trainium2 kernel for nn_LoRARouter (topk_masking).
#
# Reference computes:
#   gated  = pooled @ Wg^T            [B, D]   (B=8192, D=4096)
#   logits = gated  @ Wr^T            [B, 7]
#   probs  = softmax(logits)
#   ranks  = argsort(argsort(-rand_noise))    per [7, B, :8] group
#   out[m,b,e] = probs[b,m] > 0.5 ? (rank<2)/2 : (rank<1)/1
#
# `gated` is only consumed by the second matmul, so
#   logits = pooled @ (Wr @ Wg)^T
# removing the 275-GFLOP [B,D]x[D,D] matmul. The problem is then at the
# HBM/PE ridge: read pooled (134 MB) + Wg (67 MB once across the fleet).
#
# Design (vs the 173 us small-DMA baseline):
#  - All matmuls run as fp16 two-limb (hi + 2^11-scaled lo) products with
#    fp32 PSUM accumulation: logits = A + B/2048 where A = hi.hi and
#    B = hi.lo' + lo'.hi. Error ~2^-22 per element (measured logit err
#    3e-6 vs min decision margin 5.5e-5), but 1 cycle/row on the PE
#    instead of fp32's 4 -> PE ~65 us/core, under the ~72 us DMA floor.
#  - Host pre-formats every tensor in exact SBUF layout; inputs stream as
#    ~24 big (1-2 MB) DMAs on the sync HWDGE ring instead of ~70 small
#    ones, so the 16 SDMA engines stay fed.
#  - Wg streams first; Weff = Wr@Wg[:,shard] is computed per half-shard
#    and AllGather'd in 2 pipelined collectives triggered at ~13/~25 us,
#    hiding collective latency under the pooled_hidden stream. Each
#    core's own-shard logits run while the first gather is in flight.
#  - Select tail vectorized: per-module thresholds broadcast to
#    [128,448] once, then two full-width ops (was 56 tiny ops).
#
# Sharding (8 cores):
#  - pooled_hidden, rand_noise, output: batch-sharded (1024 rows/core)
#  - Wg: column-sharded (512 output dims/core); AllGather of WeffT.

import numpy as np

import concourse.bass as bass
import concourse.bacc as bacc
import concourse.mybir as mybir
import concourse.tile as tile
from concourse.bass_utils import run_bass_kernel_spmd

F32 = mybir.dt.float32
F16 = mybir.dt.float16
N_CORES = 8
B, D, NM, NE = 8192, 4096, 7, 8      # batch, d_model, n_modules, n_experts
BS = B // N_CORES                    # 1024 batch rows per core
SH = D // N_CORES                    # 512 Weff output dims per core
NK = D // 128                        # 32 contraction chunks of 128
NBC = BS // 128                      # 8 batch chunks of 128 per core
GRP = NM * NE                        # 56 columns per batch chunk
W = NBC * GRP                        # 448 free columns in [128, 448] tiles
SCALE = 2048.0                       # lo-limb scale 2^11
XPC = 16                             # x DMA pieces (2 chunk-slots each)
WPC = 8                              # wg DMA pieces (8 e-chunks each)

ALU = mybir.AluOpType
AF = mybir.ActivationFunctionType

_CACHE = {}
LAST_RESULTS = None  # test harness introspection


# Uniform chunk-slot order (identical on every core — the program is SPMD
# shared): half-0 chunks of all cores, then half-1. Slot s holds global
# chunk 4*b + 2*h + c with h = s//16, b = (s%16)//2, c = s%2; this matches
# the gathered WeffT block layout exactly.
CHUNK_ORDER = [4 * b + 2 * h + c for h in (0, 1) for b in range(N_CORES)
               for c in (0, 1)]


def _build_program():
    nc = bacc.Bacc(
        "TRN2", target_bir_lowering=False, debug=False, num_devices=N_CORES
    )

    # x limbs as XPC contiguous 1-MB pieces: piece p rows [128p,128(p+1)),
    # piece = 2 chunk-slots, slot s (2048 cols) = [hi:1024 b][lo:1024 b]
    xi = nc.dram_tensor("xi", [XPC * 128, 2 * 2048], F16, kind="ExternalInput")
    # wg limbs as WPC contiguous pieces: [h][ec 0..31][hi:256 d][lo:256 d]
    wgi = nc.dram_tensor("wgi", [WPC * 128, 8 * 512], F16, kind="ExternalInput")
    wrh = nc.dram_tensor("wrh", [128, NK * NM], F16, kind="ExternalInput")
    wrl = nc.dram_tensor("wrl", [128, NK * NM], F16, kind="ExternalInput")
    nzin = nc.dram_tensor("nz", [128, W], F32, kind="ExternalInput")
    cstin = nc.dram_tensor("cst", [128, W], F32, kind="ExternalInput")
    o = nc.dram_tensor("o", [128, W], F32, kind="ExternalOutput")

    # AllGather bounce per half, carrying PRE-SPLIT fp16 limbs (the split
    # runs on own data before the gather, off the critical path, so the
    # gathered weights are matmul-ready with no post-gather DVE work):
    # wfin[h][p, 0:14] = hi limbs (c=0,1), [p, 14:28] = lo limbs.
    wfin = [nc.dram_tensor(f"wfin{h}", [128, 4 * NM], F16) for h in (0, 1)]
    wfout = [
        nc.dram_tensor(
            f"wfout{h}", [N_CORES * 128, 4 * NM], F16, addr_space="Shared"
        )
        for h in (0, 1)
    ]


    with tile.TileContext(nc) as tc:
        with (
            tc.tile_pool(name="wgp", bufs=4) as wgp,
            tc.tile_pool(name="xp", bufs=XPC) as xp,
            tc.tile_pool(name="sp", bufs=1) as sp,
            tc.tile_pool(name="scr", bufs=2) as scp,
            tc.tile_pool(name="sm", bufs=16) as smp,
            tc.tile_pool(name="pw", bufs=2, space="PSUM") as pw,
            tc.tile_pool(name="pl", bufs=4, space="PSUM") as pl,
            tc.tile_pool(name="pt", bufs=2, space="PSUM") as pt,
        ):
            # ---- small input DMAs on the scalar HWDGE ring ----
            wrh_sb = sp.tile([128, NK * NM], F16, tag="wrh")
            wrl_sb = sp.tile([128, NK * NM], F16, tag="wrl")
            nz = sp.tile([128, W], F32, tag="nz")
            cstt = sp.tile([128, W], F32, tag="cst")
            nc.scalar.dma_start(wrh_sb[:], wrh[:])
            nc.scalar.dma_start(wrl_sb[:], wrl[:])
            nc.scalar.dma_start(nz[:], nzin[:])
            nc.scalar.dma_start(cstt[:], cstin[:])
            # warm the Exp activation table while the ring is idle
            warm = smp.tile([128, 1], F32, tag="warm")
            nc.scalar.activation(warm[:], nz[:, :1], AF.Exp)

            # identity for PE transposes (only [:7,:7] used)
            ident = sp.tile([128, 128], F32, tag="ident")
            from concourse.masks import make_identity
            make_identity(nc, ident[:])

            # ---- big input streams on the sync HWDGE ring (wg first) ----
            wgt = []  # piece wp: h = wp//4, ec in [8*(wp%4), 8*(wp%4)+8)
            for wp in range(WPC):
                t = wgp.tile([128, 8 * 512], F16, tag="wg")
                nc.sync.dma_start(t[:], wgi[wp * 128:(wp + 1) * 128, :])
                wgt.append(t)
            xts = []  # piece p: chunk-slots 2p, 2p+1
            for p in range(XPC):
                t = xp.tile([128, 2 * 2048], F16, tag="x")
                nc.sync.dma_start(t[:], xi[p * 128:(p + 1) * 128, :])
                xts.append(t)

            # ---- Weff halves: [7,256] = Wr @ Wg[:, own 256 cols], two-limb
            # chains A (hi.hi) and B (hi.lo' + lo'.hi), then AllGather ----
            wfg16 = []     # gathered limbs (hi, lo) per half
            for h in (0, 1):
                wA = pw.tile([7, 256], F32, tag="pw", name=f"wA{h}")
                wB = pw.tile([7, 256], F32, tag="pw", name=f"wB{h}")
                for ec in range(NK):
                    wp = h * 4 + ec // 8
                    base = (ec % 8) * 512
                    ghi = wgt[wp][:, base:base + 256]
                    glo = wgt[wp][:, base + 256:base + 512]
                    rsl = slice(ec * NM, (ec + 1) * NM)
                    st, sp_ = (ec == 0), (ec == NK - 1)
                    nc.tensor.matmul(wA[:], wrh_sb[:, rsl], ghi, start=st, stop=sp_)
                    nc.tensor.matmul(wB[:], wrl_sb[:, rsl], ghi, start=st, stop=False)
                    nc.tensor.matmul(wB[:], wrh_sb[:, rsl], glo, start=False, stop=sp_)
                # combine: wf = wA + wB/SCALE  [7, 256] f32
                tb = scp.tile([7, 256], F32, tag="scr2")
                nc.vector.tensor_scalar_mul(tb[:], wB[:], 1.0 / SCALE)
                wf = sp.tile([7, 256], F32, tag=f"wf{h}")
                nc.vector.tensor_tensor(wf[:], tb[:], wA[:], ALU.add)
                # transpose both 128-col blocks to d-major [128, 14]
                wsh = sp.tile([128, 2 * NM], F32, tag=f"wsh{h}")
                for c in (0, 1):
                    tr = pt.tile([128, NM], F32, tag="pt")
                    nc.tensor.transpose(
                        tr[:], wf[:, c * 128:(c + 1) * 128], ident[:7, :7]
                    )
                    nc.vector.tensor_copy(wsh[:, c * NM:(c + 1) * NM], tr[:])
                # split own limbs BEFORE the gather (hidden under the DMA
                # stream), then bounce both limb sets for this half
                whi = sp.tile([128, 2 * NM], F16, tag=f"whi{h}")
                wlo = sp.tile([128, 2 * NM], F16, tag=f"wlo{h}")
                hi32 = scp.tile([128, 2 * NM], F32, tag="scr4")
                dif = scp.tile([128, 2 * NM], F32, tag="scr4")
                nc.vector.tensor_copy(whi[:], wsh[:])
                nc.vector.tensor_copy(hi32[:], whi[:])
                nc.vector.tensor_tensor(dif[:], wsh[:], hi32[:], ALU.subtract)
                nc.vector.tensor_scalar_mul(wlo[:], dif[:], SCALE)
                nc.scalar.dma_start(wfin[h][:, :2 * NM], whi[:])
                nc.scalar.dma_start(wfin[h][:, 2 * NM:], wlo[:])

            # pipelined AllGathers (both triggers before either load-back
            # so a slow first gather can't delay the second trigger)
            for h in (0, 1):
                nc.gpsimd.collective_compute(
                    "AllGather",
                    ALU.bypass,
                    replica_groups=[list(range(N_CORES))],
                    ins=[wfin[h][:]],
                    outs=[wfout[h][:]],
                )
            # load back matmul-ready limbs: wfg[h][p, j*28 + (hi|lo)*14 + c*7+m].
            # On the scalar HWDGE ring: the rearrange generates ~1024 tiny
            # descriptors, which SWDGE's Q7 software emits serially (~4-6 us)
            # but HWDGE RTL emits at line rate — this load sits right on the
            # gather -> first-matmul critical path.
            for h in (0, 1):
                wfg = sp.tile([128, N_CORES * 4 * NM], F16, tag=f"wfg{h}")
                nc.scalar.dma_start(
                    wfg[:].rearrange("p (j f) -> p j f", j=N_CORES),
                    wfout[h][:].rearrange("(j p) f -> p j f", p=128),
                )
                wfg16.append(wfg)

            # ---- expert ranks from rand_noise (overlaps the DMA phase on
            # DVE). r[e] = #{j<e: v_j >= v_e} + #{j>e: v_j > v_e}; acc
            # starts at cst[e] = 7-e; each offset's comparison adds 1 at
            # the A position and subtracts 1 at the B position. ----
            acc = sp.tile([128, W], F32, tag="acc")
            nc.vector.tensor_copy(acc[:], cstt[:])
            nz_r = nz[:].rearrange("p (c m e) -> p c m e", m=NM, e=NE)
            acc_r = acc[:].rearrange("p (c m e) -> p c m e", m=NM, e=NE)
            for off in range(1, NE):
                wdt = NE - off
                scr = scp.tile([128, NBC * NM * 7], F32, tag="scr")
                scr_v = scr[:, : NBC * NM * wdt].rearrange(
                    "p (c m e) -> p c m e", m=NM, e=wdt
                )
                nc.vector.tensor_tensor(
                    scr_v, nz_r[:, :, :, 0:wdt], nz_r[:, :, :, off:NE], ALU.is_ge
                )
                nc.vector.tensor_tensor(
                    acc_r[:, :, :, off:NE], acc_r[:, :, :, off:NE], scr_v, ALU.add
                )
                nc.vector.tensor_tensor(
                    acc_r[:, :, :, 0:wdt], acc_r[:, :, :, 0:wdt], scr_v, ALU.subtract
                )

            # ---- logitsT: two-limb chains per batch half, accumulated
            # over the 32 chunk-slots in stream order ----
            psA = [pl.tile([7, 512], F32, tag="pl", name=f"pA{bh}") for bh in (0, 1)]
            psB = [pl.tile([7, 512], F32, tag="pl", name=f"pB{bh}") for bh in (0, 1)]

            def slot_lhsT(s):
                # weight limb slices [128, 7] for chunk-slot s; slot order
                # matches the gathered block layout (see CHUNK_ORDER)
                h = s // 16
                j, c = (s % 16) // 2, s % 2
                wfg = wfg16[h]
                base = j * 4 * NM
                hi_sl = slice(base + c * NM, base + (c + 1) * NM)
                lo_sl = slice(base + 2 * NM + c * NM, base + 2 * NM + (c + 1) * NM)
                return wfg[:, hi_sl], wfg[:, lo_sl]

            # Hybrid block order: both batch-halves' h0-slot blocks run
            # first (PE reaches the half-1 gather dependency at +20.8 us,
            # the full window), then bh0's h1 block STOPS its PSUM chains
            # ~10.4 us before the last matmul, so bh0's combine/softmax/
            # select tail overlaps under bh1's final block. Only bh1's
            # half-tail stays exposed after the last matmul.
            logT = sp.tile([7, BS], F32, tag="logT")
            thr = sp.tile([128, NBC * NM], F32, tag="thr")
            threp = sp.tile([128, W], F32, tag="threp")
            valrep = sp.tile([128, W], F32, tag="valrep")
            outt = sp.tile([128, W], F32, tag="outt")
            thr_v = thr[:].rearrange("p (c m) -> p c m", m=NM)
            threp_v = threp[:].rearrange("p (c m e) -> p c m e", m=NM, e=NE)
            HBC = NBC // 2          # 4 batch chunks per half
            HW_ = W // 2            # 224 select columns per half

            def mm_block(bh, s_lo, s_hi):
                for s in range(s_lo, s_hi):
                    xt = xts[s // 2]
                    xb = (s % 2) * 2048
                    whi, wlo = slot_lhsT(s)
                    st, sp_ = (s == 0), (s == NK - 1)
                    xhi = xt[:, xb + bh * 512: xb + bh * 512 + 512]
                    xlo = xt[:, xb + 1024 + bh * 512:
                             xb + 1024 + bh * 512 + 512]
                    nc.tensor.matmul(psA[bh][:], whi, xhi, start=st, stop=sp_)
                    nc.tensor.matmul(psB[bh][:], wlo, xhi, start=st, stop=False)
                    nc.tensor.matmul(psB[bh][:], whi, xlo, start=False, stop=sp_)

            def tail_half(bh):
                # combine: logT half = A + B/SCALE
                tb = scp.tile([7, 512], F32, tag="scr5")
                nc.vector.tensor_scalar_mul(tb[:], psB[bh][:], 1.0 / SCALE)
                nc.vector.tensor_tensor(
                    logT[:, bh * 512:(bh + 1) * 512], tb[:], psA[bh][:], ALU.add
                )
                # softmax>0.5 -> thr in {1,2} for this half's batch chunks
                for bc in range(bh * HBC, (bh + 1) * HBC):
                    plt = pt.tile([128, NM], F32, tag="pt")
                    nc.tensor.transpose(
                        plt[:], logT[:, bc * 128:(bc + 1) * 128], ident[:7, :7]
                    )
                    # |logits| < 9: exp cannot overflow fp32, skip max-sub
                    ssum = smp.tile([128, 1], F32, tag="ssum")
                    shalf = smp.tile([128, 1], F32, tag="shalf")
                    expt = smp.tile([128, NM], F32, tag="expt")
                    nc.scalar.activation(
                        expt[:], plt[:], AF.Exp, accum_out=ssum[:]
                    )
                    nc.vector.tensor_scalar_mul(shalf[:], ssum[:], 0.5)
                    nc.vector.tensor_scalar(
                        out=thr[:, bc * NM:(bc + 1) * NM], in0=expt[:],
                        scalar1=shalf[:], scalar2=1.0, op0=ALU.is_gt, op1=ALU.add,
                    )
                # select for this half (all on DVE; gpsimd runs these
                # strided ops 4-5x slower)
                cs = slice(bh * HBC, (bh + 1) * HBC)
                ws = slice(bh * HW_, (bh + 1) * HW_)
                for e in range(NE):
                    nc.vector.tensor_copy(threp_v[:, cs, :, e], thr_v[:, cs, :])
                nc.vector.tensor_scalar(
                    out=valrep[:, ws], in0=threp[:, ws], scalar1=-0.5,
                    scalar2=1.5, op0=ALU.mult, op1=ALU.add,
                )
                nc.vector.tensor_tensor(
                    outt[:, ws], acc[:, ws], threp[:, ws], ALU.is_lt
                )
                nc.vector.tensor_tensor(
                    outt[:, ws], outt[:, ws], valrep[:, ws], ALU.mult
                )

            mm_block(0, 0, NK // 2)
            mm_block(1, 0, NK // 2)
            mm_block(0, NK // 2, NK)
            tail_half(0)
            mm_block(1, NK // 2, NK)
            tail_half(1)
            nc.scalar.dma_start(o[:], outt[:])

    nc.compile()
    return nc


def _get_program():
    if "nc" not in _CACHE:
        _CACHE["nc"] = _build_program()
    return _CACHE["nc"]


def _split16(x):
    hi = x.astype(np.float16)
    lo = ((x - hi.astype(np.float32)) * SCALE).astype(np.float16)
    return hi, lo


def _const_input():
    base = (7.0 - np.arange(NE, dtype=np.float32))
    return np.ascontiguousarray(
        np.broadcast_to(np.tile(base, NBC * NM), (128, W))
    )


def _prep_core(i, ph, wg_full, rn):
    bsl = slice(i * BS, (i + 1) * BS)
    xc = np.ascontiguousarray(ph[bsl].T).reshape(NK, 128, BS)[CHUNK_ORDER]
    hi, lo = _split16(xc)                          # [32, 128, 1024] each
    xi = np.concatenate([hi, lo], axis=2)          # [32, 128, 2048]
    # piece-contiguous: piece p (2 slots) occupies rows [128p, 128(p+1))
    xi = np.ascontiguousarray(
        xi.reshape(XPC, 2, 128, 2048).transpose(0, 2, 1, 3)
        .reshape(XPC * 128, 2 * 2048)
    )

    esl = slice(i * SH, (i + 1) * SH)
    wgc = np.ascontiguousarray(wg_full[:, esl]).reshape(NK, 128, SH)
    segs = []
    for h in (0, 1):
        seg = wgc[:, :, h * 256:(h + 1) * 256]     # [32, 128, 256]
        shi, slo = _split16(seg)
        segs.append(np.concatenate([shi, slo], axis=2))   # [32, 128, 512]
    wgi = np.stack(segs, 0)                        # [2, 32, 128, 512]
    # piece-contiguous: piece wp = (h, 8-ec block) occupies rows [128wp, ..)
    wgi = np.ascontiguousarray(
        wgi.reshape(2, 4, 8, 128, 512).transpose(0, 1, 3, 2, 4)
        .reshape(WPC * 128, 8 * 512)
    )

    # nz[p, c*56 + m*8 + e] = rn[m, 1024*i + 128*c + p, e]
    nz_i = np.ascontiguousarray(
        rn[:, bsl, :].transpose(1, 0, 2)
        .reshape(NBC, 128, GRP).transpose(1, 0, 2).reshape(128, W)
    )
    return xi, wgi, nz_i


def kernel(pooled_hidden, Wg, Wr, rand_noise):
    global LAST_RESULTS
    ph = np.asarray(pooled_hidden, dtype=np.float32)
    wg_full = np.asarray(Wg, dtype=np.float32)
    wr = np.asarray(Wr, dtype=np.float32)
    rn = np.asarray(rand_noise, dtype=np.float32)

    nc = _get_program()
    cst = _const_input()

    # Wr in SBUF layout [p, ec*7+m] = Wr[m, ec*128+p], fp16 limbs
    wrt = np.ascontiguousarray(
        wr.T.reshape(NK, 128, NM).transpose(1, 0, 2).reshape(128, NK * NM)
    )
    wrh_full, wrl_full = _split16(wrt)
    wrh_full = np.ascontiguousarray(wrh_full)
    wrl_full = np.ascontiguousarray(wrl_full)

    in_maps = []
    for i in range(N_CORES):
        xi, wgi, nz_i = _prep_core(i, ph, wg_full, rn)
        in_maps.append(
            {"xi": xi, "wgi": wgi, "wrh": wrh_full, "wrl": wrl_full,
             "nz": nz_i, "cst": cst}
        )

    res = run_bass_kernel_spmd(nc, in_maps, list(range(N_CORES)))
    LAST_RESULTS = res

    out = np.empty((NM, B, NE), dtype=np.float32)
    for i, r in enumerate(res.results):
        oc = r["o"]  # [128, 448]
        out[:, i * BS:(i + 1) * BS, :] = (
            oc.reshape(128, NBC, NM, NE).transpose(2, 1, 0, 3).reshape(NM, BS, NE)
        )
    return out
